# revision 1
# baseline (speedup 1.0000x reference)
"""Trainium2 Bass kernel for the windowed 3-channel MLP (dense_mlp).

Reference computation (B=8192):
  x [B, 6144] -> view [B, 3, 2048]
  16 overlapping windows/channel (len 256, stride 119)
  h[b,c,w,:] = win @ W1[c,w] + b1[c,w]          # [B,3,16,64]
  h = mean over c                               # [B,16,64]
  g[b,grp]   = h-grp(4 windows=256) @ W2[grp] + b2   # [B,4,64]
  out        = g.reshape(B,256) @ W3 + b3       # [B,255]

Strategy: pure data parallelism over 8 cores (B/8 = 1024 rows each).
Compute in fp16 (accumulation in f32 PSUM); x is shipped to device DRAM as
fp16 so the DMA-transpose (xbar) can read it directly from DRAM into
feature-major SBUF tiles — no on-chip staging, casting, or SBUF->SBUF pass.

On-device per core:
  - 4 batch chunks (256/384/256/128 rows; small last chunk shrinks the serial
    tail); per chunk one DRAM->SBUF xbar transpose produces
    xT [128k, 48 ktiles, nb].
  - Layer 1 as banded matmuls over 128-aligned k-tiles with host-packed
    zero-padded weight blocks (channel-mean folded into PSUM accumulation,
    1/3 folded into W1).
  - Layers 2/3 stay feature-major; layer 3 uses gT as lhsT so the output
    comes out batch-major for a contiguous DMA out.
"""

import sys

sys.path.insert(0, "/opt/trn_rl_repo")

import numpy as np

import concourse.bass as bass
import concourse.mybir as mybir
import concourse.tile as tile
from concourse import bacc
from concourse.bass_utils import run_bass_kernel_spmd

P = 128
N_CORES = 8
B_FULL = 8192
B_SHARD = B_FULL // N_CORES          # 1024
CH_LEN = 2048
N_CH = 3
K_FULL = N_CH * CH_LEN               # 6144
N_WIN = 16
WIN = 256
STRIDE = 119
N_PAIR = 8                           # window pairs (2 windows x 64 = 128 feats)
KT_CH = CH_LEN // P                  # 16 k-tiles per channel
KT_ALL = K_FULL // P                 # 48
NB = 384                             # max batch chunk (matmul free dim)
CHUNKS = [256, 384, 256, 128]        # batch chunk sizes (sum = B_SHARD)
assert sum(CHUNKS) == B_SHARD
N_OUT = 255

def _pair_tiles(m):
    """k-tiles of one channel that intersect window pair m (rows 238m..238m+374)."""
    lo = (2 * STRIDE * m) // P
    hi = (2 * STRIDE * m + 2 * STRIDE + WIN - 2 - STRIDE) // P  # (238m+374)//128
    return list(range(lo, min(hi, KT_CH - 1) + 1))

# Block order for layer-1 packed weights: for m, for c, for t.
BLOCKS = [(m, c, t) for m in range(N_PAIR) for c in range(N_CH) for t in _pair_tiles(m)]
BLK_IDX = {key: i for i, key in enumerate(BLOCKS)}
N_BLK = len(BLOCKS)                  # 90


def _pack_weights(W1, b1, W2, b2, W3, b3):
    """Host-side packing of the tiny weight tensors into device layouts."""
    W1 = np.asarray(W1, dtype=np.float32)
    ki = np.arange(P)[:, None]                    # tile-local k row
    j = np.arange(P)[None, :]                     # pair-local output feature
    w_off = j // 64                               # window within pair
    n = j % 64

    w1p = np.zeros((N_BLK, P, P), dtype=np.float32)
    for i, (m, c, t) in enumerate(BLOCKS):
        w = 2 * m + w_off                         # [1,128] window index
        koff = P * t + ki - STRIDE * w            # [128,128] k within window
        mask = (koff >= 0) & (koff < WIN)
        w1p[i] = np.where(
            mask, W1[c, w, np.clip(koff, 0, WIN - 1), n] / 3.0, 0.0
        )
    # device layout: [P(ki), N_BLK * P(j)] contiguous per partition
    w1sb = np.ascontiguousarray(
        w1p.transpose(1, 0, 2).reshape(P, N_BLK * P)
    ).astype(np.float16)

    # W2 [4,256,64] -> pieces [g,p][128,64] -> [P, 8, 64]
    w2p = np.asarray(W2, dtype=np.float32).reshape(4, 2, P, 64)
    w2sb = np.ascontiguousarray(
        w2p.transpose(2, 0, 1, 3).reshape(P, 8 * 64)
    ).astype(np.float16)

    # W3 [256,255] -> [P, 2, 255]
    w3p = np.asarray(W3, dtype=np.float32).reshape(2, P, N_OUT)
    w3sb = np.ascontiguousarray(
        w3p.transpose(1, 0, 2).reshape(P, 2 * N_OUT)
    ).astype(np.float16)

    # biases (per-partition layouts)
    b1m = np.asarray(b1, dtype=np.float32).mean(axis=0)        # [16,64]
    b1t = np.ascontiguousarray(b1m.reshape(N_PAIR, P).T)       # [128, 8]
    b2t = np.ascontiguousarray(np.asarray(b2, dtype=np.float32).T)  # [64, 4]
    b3t = np.ascontiguousarray(
        np.broadcast_to(np.asarray(b3, dtype=np.float32), (P, N_OUT))
    )                                                          # [128, 255]
    return w1sb, w2sb, w3sb, b1t, b2t, b3t


def build_kernel(reps=1, has_bias=False):
    nc = bacc.Bacc("TRN2", target_bir_lowering=False, debug=False,
                   num_devices=N_CORES)
    f16 = mybir.dt.float16
    f32 = mybir.dt.float32

    x_ext = nc.declare_dram_parameter("x", [B_SHARD, K_FULL], f16, isOutput=False)
    w1_ext = nc.declare_dram_parameter("w1", [P, N_BLK * P], f16, isOutput=False)
    w2_ext = nc.declare_dram_parameter("w2", [P, 8 * 64], f16, isOutput=False)
    w3_ext = nc.declare_dram_parameter("w3", [P, 2 * N_OUT], f16, isOutput=False)
    b1_ext = nc.declare_dram_parameter("b1t", [P, N_PAIR], f32, isOutput=False)
    b2_ext = nc.declare_dram_parameter("b2t", [64, 4], f32, isOutput=False)
    b3_ext = nc.declare_dram_parameter("b3t", [P, N_OUT], f32, isOutput=False)
    out_ext = nc.declare_dram_parameter("out", [B_SHARD, N_OUT], f32, isOutput=True)

    with tile.TileContext(nc) as tc:
        with (
            tc.tile_pool(name="wpool", bufs=1) as wpool,
            tc.tile_pool(name="xt", bufs=3) as xt_pool,
            tc.tile_pool(name="hp", bufs=10) as hp_pool,
            tc.tile_pool(name="gt", bufs=2) as gt_pool,
            tc.tile_pool(name="osb", bufs=3) as out_pool,
            tc.tile_pool(name="ps1", bufs=4, space="PSUM") as ps1_pool,
            tc.tile_pool(name="ps2", bufs=2, space="PSUM") as ps2_pool,
            tc.tile_pool(name="ps3", bufs=2, space="PSUM") as ps3_pool,
        ):
            w1sb = wpool.tile([P, N_BLK, P], f16)
            nc.scalar.dma_start(out=w1sb[:], in_=w1_ext.rearrange("p (b j) -> p b j", j=P))
            w2sb = wpool.tile([P, 8, 64], f16)
            nc.scalar.dma_start(out=w2sb[:], in_=w2_ext.rearrange("p (b j) -> p b j", j=64))
            w3sb = wpool.tile([P, 2, N_OUT], f16)
            nc.scalar.dma_start(out=w3sb[:], in_=w3_ext.rearrange("p (b j) -> p b j", j=N_OUT))
            b1sb = wpool.tile([P, N_PAIR], f32)
            nc.scalar.dma_start(out=b1sb[:], in_=b1_ext[:])
            b2sb = wpool.tile([64, 4], f32)
            nc.scalar.dma_start(out=b2sb[:], in_=b2_ext[:])
            b3sb = wpool.tile([P, N_OUT], f32)
            nc.scalar.dma_start(out=b3sb[:], in_=b3_ext[:])

            import contextlib
            loop_cm = tc.For_i(0, reps, 1) if reps > 1 else contextlib.nullcontext()
            with loop_cm:
                _kernel_body(nc, tc, locals(), has_bias)

    nc.compile()
    return nc


def _kernel_body(nc, tc, env, has_bias):
    x_ext = env["x_ext"]
    out_ext = env["out_ext"]
    w1sb, w2sb, w3sb = env["w1sb"], env["w2sb"], env["w3sb"]
    b1sb, b2sb, b3sb = env["b1sb"], env["b2sb"], env["b3sb"]
    xt_pool = env["xt_pool"]
    hp_pool, gt_pool, out_pool = env["hp_pool"], env["gt_pool"], env["out_pool"]
    ps1_pool, ps2_pool, ps3_pool = env["ps1_pool"], env["ps2_pool"], env["ps3_pool"]
    f16 = mybir.dt.float16
    f32 = mybir.dt.float32

    b0 = 0
    for ch, nb in enumerate(CHUNKS):
        # one xbar transpose: x[b0:b0+nb, :] (DRAM, fp16) -> [128k, 48, nb]
        xt_t = xt_pool.tile([P, KT_ALL, NB], f16, name="xtt")
        xt = xt_t[:, :, :nb]
        nc.sync.dma_start(out=xt[:], in_=x_ext[b0:b0 + nb, :], transpose=True)

        # ---- layer 1: banded matmuls per window pair ----
        hps = {}
        for m in range(N_PAIR):
            ps_t = ps1_pool.tile([P, NB], f32, name="ps1t")
            ps = ps_t[:, :nb]
            mm_list = [(c, t) for c in range(N_CH) for t in _pair_tiles(m)]
            for i, (c, t) in enumerate(mm_list):
                nc.tensor.matmul(
                    ps[:],
                    w1sb[:, BLK_IDX[(m, c, t)], :],
                    xt[:, c * KT_CH + t, :],
                    start=(i == 0),
                    stop=(i == len(mm_list) - 1),
                )
            hp_t = hp_pool.tile([P, NB], f16, name="hpt")
            hp = hp_t[:, :nb]
            if has_bias:
                nc.vector.tensor_scalar_add(hp[:], ps[:], b1sb[:, m:m + 1])
            else:
                nc.vector.tensor_copy(out=hp[:], in_=ps[:])
            hps[m] = hp

        # ---- layer 2: 4 groups of 4 windows ----
        gt_t = gt_pool.tile([P, 2, NB], f16, name="gtt")
        gt = gt_t[:, :, :nb]
        for g in range(4):
            ps2_t = ps2_pool.tile([64, NB], f32, name="ps2t")
            ps2 = ps2_t[:, :nb]
            for piece in range(2):
                nc.tensor.matmul(
                    ps2[:],
                    w2sb[:, 2 * g + piece, :],
                    hps[2 * g + piece][:],
                    start=(piece == 0),
                    stop=(piece == 1),
                )
            lo = 64 * (g % 2)
            if has_bias:
                nc.vector.tensor_scalar_add(
                    gt[lo:lo + 64, g // 2], ps2[:], b2sb[:, g:g + 1],
                )
            else:
                nc.vector.tensor_copy(out=gt[lo:lo + 64, g // 2], in_=ps2[:])

        # ---- layer 3: back to batch-major ----
        osb_t = out_pool.tile([P, NB // P, N_OUT], f32, name="osbt")
        osb = osb_t[:, :nb // P]
        for js in range(nb // P):
            ps3 = ps3_pool.tile([P, N_OUT], f32)
            for piece in range(2):
                nc.tensor.matmul(
                    ps3[:],
                    gt[:, piece, js * P:(js + 1) * P],
                    w3sb[:, piece, :],
                    start=(piece == 0),
                    stop=(piece == 1),
                )
            if has_bias:
                nc.vector.tensor_tensor(
                    osb[:, js], ps3[:], b3sb[:], mybir.AluOpType.add,
                )
            else:
                nc.vector.tensor_copy(out=osb[:, js], in_=ps3[:])
        nc.scalar.dma_start(
            out=out_ext[b0:b0 + nb, :].rearrange("(j p) n -> p j n", p=P),
            in_=osb[:],
        )
        b0 += nb


_CACHED_NC = None


def _prep_in_maps(x, W1, b1, W2, b2, W3, b3):
    x16 = np.asarray(x, dtype=np.float16)
    w1sb, w2sb, w3sb, b1t, b2t, b3t = _pack_weights(W1, b1, W2, b2, W3, b3)
    in_maps = []
    for i in range(N_CORES):
        in_maps.append({
            "x": x16[i * B_SHARD:(i + 1) * B_SHARD],
            "w1": w1sb,
            "w2": w2sb,
            "w3": w3sb,
            "b1t": b1t,
            "b2t": b2t,
            "b3t": b3t,
        })
    return in_maps


_CACHED_BIAS_NC = None


def kernel(x, W1, b1, W2, b2, W3, b3):
    global _CACHED_NC, _CACHED_BIAS_NC
    has_bias = bool(
        np.any(np.asarray(b1)) or np.any(np.asarray(b2)) or np.any(np.asarray(b3))
    )
    if has_bias:
        if _CACHED_BIAS_NC is None:
            _CACHED_BIAS_NC = build_kernel(has_bias=True)
        nc = _CACHED_BIAS_NC
    else:
        if _CACHED_NC is None:
            _CACHED_NC = build_kernel()
        nc = _CACHED_NC
    in_maps = _prep_in_maps(x, W1, b1, W2, b2, W3, b3)
    last_err = None
    for attempt in range(3):
        try:
            res = run_bass_kernel_spmd(nc, in_maps, core_ids=list(range(N_CORES)))
            break
        except Exception as e:  # transient device/axon failures
            last_err = e
            if attempt == 2:
                raise
            import time as _time
            _time.sleep(20.0)
    return np.concatenate([res.results[i]["out"] for i in range(N_CORES)], axis=0)



# revision 3
# speedup vs baseline: 1.3786x; 1.3786x over previous
"""Trainium2 Bass kernel for the windowed 3-channel MLP (dense_mlp).

Reference computation (B=8192):
  x [B, 6144] -> view [B, 3, 2048]
  16 overlapping windows/channel (len 256, stride 119)
  h[b,c,w,:] = win @ W1[c,w] + b1[c,w]          # [B,3,16,64]
  h = mean over c                               # [B,16,64]
  g[b,grp]   = h-grp(4 windows=256) @ W2[grp] + b2   # [B,4,64]
  out        = g.reshape(B,256) @ W3 + b3       # [B,255]

Strategy: pure data parallelism over 8 cores (B/8 = 1024 rows each).
Compute in fp16 (accumulation in f32 PSUM); x is shipped to device DRAM as
fp16 so the DMA-transpose (xbar) can read it directly from DRAM into
feature-major SBUF tiles — no on-chip staging, casting, or SBUF->SBUF pass.

On-device per core:
  - 4 batch chunks (256/384/256/128 rows; small last chunk shrinks the serial
    tail); per chunk one DRAM->SBUF xbar transpose produces
    xT [128k, 48 ktiles, nb].
  - Layer 1 as banded matmuls over 128-aligned k-tiles with host-packed
    zero-padded weight blocks (channel-mean folded into PSUM accumulation,
    1/3 folded into W1).
  - Layers 2/3 stay feature-major; layer 3 uses gT as lhsT so the output
    comes out batch-major for a contiguous DMA out.
"""

import sys

sys.path.insert(0, "/opt/trn_rl_repo")

import numpy as np

import concourse.bass as bass
import concourse.mybir as mybir
import concourse.tile as tile
from concourse import bacc
from concourse.bass_utils import run_bass_kernel_spmd

P = 128
N_CORES = 8
B_FULL = 8192
B_SHARD = B_FULL // N_CORES          # 1024
CH_LEN = 2048
N_CH = 3
K_FULL = N_CH * CH_LEN               # 6144
N_WIN = 16
WIN = 256
STRIDE = 119
N_PAIR = 8                           # window pairs (2 windows x 64 = 128 feats)
KT_CH = CH_LEN // P                  # 16 k-tiles per channel
KT_ALL = K_FULL // P                 # 48
NB = 384                             # max batch chunk (matmul free dim)
CHUNKS = [256, 256, 256, 256]        # batch chunk sizes (sum = B_SHARD)
assert sum(CHUNKS) == B_SHARD
N_OUT = 255

def _pair_tiles(m):
    """k-tiles of one channel that intersect window pair m (rows 238m..238m+374)."""
    lo = (2 * STRIDE * m) // P
    hi = (2 * STRIDE * m + 2 * STRIDE + WIN - 2 - STRIDE) // P  # (238m+374)//128
    return list(range(lo, min(hi, KT_CH - 1) + 1))

# Block order for layer-1 packed weights: for m, for c, for t.
BLOCKS = [(m, c, t) for m in range(N_PAIR) for c in range(N_CH) for t in _pair_tiles(m)]
BLK_IDX = {key: i for i, key in enumerate(BLOCKS)}
N_BLK = len(BLOCKS)                  # 90


def _pack_weights(W1, b1, W2, b2, W3, b3):
    """Host-side packing of the tiny weight tensors into device layouts."""
    W1 = np.asarray(W1, dtype=np.float32)
    ki = np.arange(P)[:, None]                    # tile-local k row
    j = np.arange(P)[None, :]                     # pair-local output feature
    w_off = j // 64                               # window within pair
    n = j % 64

    w1p = np.zeros((N_BLK, P, P), dtype=np.float32)
    for i, (m, c, t) in enumerate(BLOCKS):
        w = 2 * m + w_off                         # [1,128] window index
        koff = P * t + ki - STRIDE * w            # [128,128] k within window
        mask = (koff >= 0) & (koff < WIN)
        w1p[i] = np.where(
            mask, W1[c, w, np.clip(koff, 0, WIN - 1), n] / 3.0, 0.0
        )
    # device layout: [P(ki), N_BLK * P(j)] contiguous per partition
    w1sb = np.ascontiguousarray(
        w1p.transpose(1, 0, 2).reshape(P, N_BLK * P)
    ).astype(np.float16)

    # W2 [4,256,64] -> pieces [g,p][128,64] -> [P, 8, 64]
    w2p = np.asarray(W2, dtype=np.float32).reshape(4, 2, P, 64)
    w2sb = np.ascontiguousarray(
        w2p.transpose(2, 0, 1, 3).reshape(P, 8 * 64)
    ).astype(np.float16)

    # W3 [256,255] -> [P, 2, 255]
    w3p = np.asarray(W3, dtype=np.float32).reshape(2, P, N_OUT)
    w3sb = np.ascontiguousarray(
        w3p.transpose(1, 0, 2).reshape(P, 2 * N_OUT)
    ).astype(np.float16)

    # biases (per-partition layouts)
    b1m = np.asarray(b1, dtype=np.float32).mean(axis=0)        # [16,64]
    b1t = np.ascontiguousarray(b1m.reshape(N_PAIR, P).T)       # [128, 8]
    b2t = np.ascontiguousarray(np.asarray(b2, dtype=np.float32).T)  # [64, 4]
    b3t = np.ascontiguousarray(
        np.broadcast_to(np.asarray(b3, dtype=np.float32), (P, N_OUT))
    )                                                          # [128, 255]
    return w1sb, w2sb, w3sb, b1t, b2t, b3t


def build_kernel(reps=1, has_bias=False):
    nc = bacc.Bacc("TRN2", target_bir_lowering=False, debug=False,
                   num_devices=N_CORES)
    f16 = mybir.dt.float16
    f32 = mybir.dt.float32

    x_ext = nc.declare_dram_parameter("x", [B_SHARD, K_FULL], f16, isOutput=False)
    w1_ext = nc.declare_dram_parameter("w1", [P, N_BLK * P], f16, isOutput=False)
    w2_ext = nc.declare_dram_parameter("w2", [P, 8 * 64], f16, isOutput=False)
    w3_ext = nc.declare_dram_parameter("w3", [P, 2 * N_OUT], f16, isOutput=False)
    b1_ext = nc.declare_dram_parameter("b1t", [P, N_PAIR], f32, isOutput=False)
    b2_ext = nc.declare_dram_parameter("b2t", [64, 4], f32, isOutput=False)
    b3_ext = nc.declare_dram_parameter("b3t", [P, N_OUT], f32, isOutput=False)
    out_ext = nc.declare_dram_parameter("out", [B_SHARD, N_OUT], f32, isOutput=True)

    with tile.TileContext(nc) as tc:
        with (
            tc.tile_pool(name="wpool", bufs=1) as wpool,
            tc.tile_pool(name="xt", bufs=3) as xt_pool,
            tc.tile_pool(name="hp", bufs=10) as hp_pool,
            tc.tile_pool(name="gt", bufs=2) as gt_pool,
            tc.tile_pool(name="osb", bufs=3) as out_pool,
            tc.tile_pool(name="ps1", bufs=4, space="PSUM") as ps1_pool,
            tc.tile_pool(name="ps2", bufs=2, space="PSUM") as ps2_pool,
            tc.tile_pool(name="ps3", bufs=2, space="PSUM") as ps3_pool,
        ):
            w1sb = wpool.tile([P, N_BLK, P], f16)
            nc.scalar.dma_start(out=w1sb[:], in_=w1_ext.rearrange("p (b j) -> p b j", j=P))
            w2sb = wpool.tile([P, 8, 64], f16)
            nc.scalar.dma_start(out=w2sb[:], in_=w2_ext.rearrange("p (b j) -> p b j", j=64))
            w3sb = wpool.tile([P, 2, N_OUT], f16)
            nc.scalar.dma_start(out=w3sb[:], in_=w3_ext.rearrange("p (b j) -> p b j", j=N_OUT))
            b1sb = wpool.tile([P, N_PAIR], f32)
            nc.scalar.dma_start(out=b1sb[:], in_=b1_ext[:])
            b2sb = wpool.tile([64, 4], f32)
            nc.scalar.dma_start(out=b2sb[:], in_=b2_ext[:])
            b3sb = wpool.tile([P, N_OUT], f32)
            nc.scalar.dma_start(out=b3sb[:], in_=b3_ext[:])

            # chunk-0/1 x tiles are software-pipelined across reps:
            # allocated once, loaded in a prologue, and re-loaded at the END
            # of each body so the next iteration starts with both resident.
            xt0 = wpool.tile([P, KT_ALL, CHUNKS[0]], mybir.dt.float16,
                             name="xt0")
            nc.sync.dma_start(out=xt0[:], in_=x_ext[0:CHUNKS[0], :],
                              transpose=True)
            xt1 = wpool.tile([P, KT_ALL, CHUNKS[1]], mybir.dt.float16,
                             name="xt1")
            nc.sync.dma_start(
                out=xt1[:],
                in_=x_ext[CHUNKS[0]:CHUNKS[0] + CHUNKS[1], :],
                transpose=True)
            import contextlib
            loop_cm = (tc.For_i(0, reps, 1,
                                hint_engines=(mybir.EngineType.PE,))
                       if reps > 1 else contextlib.nullcontext())
            with loop_cm:
                _kernel_body(nc, tc, locals(), has_bias, prefetch=reps > 1)

    nc.compile()
    return nc


def _kernel_body(nc, tc, env, has_bias, prefetch=False):
    xt0 = env["xt0"]
    xt1 = env["xt1"]
    x_ext = env["x_ext"]
    out_ext = env["out_ext"]
    w1sb, w2sb, w3sb = env["w1sb"], env["w2sb"], env["w3sb"]
    b1sb, b2sb, b3sb = env["b1sb"], env["b2sb"], env["b3sb"]
    xt_pool = env["xt_pool"]
    hp_pool, gt_pool, out_pool = env["hp_pool"], env["gt_pool"], env["out_pool"]
    ps1_pool, ps2_pool, ps3_pool = env["ps1_pool"], env["ps2_pool"], env["ps3_pool"]
    f16 = mybir.dt.float16
    f32 = mybir.dt.float32

    b0 = 0
    for ch, nb in enumerate(CHUNKS):
        if ch == 0:
            xt = xt0[:, :, :nb]          # preloaded (prologue / prev iter)
        elif ch == 1:
            xt = xt1[:, :, :nb]          # preloaded (prologue / prev iter)
        else:
            # one xbar transpose: x[b0:b0+nb, :] (DRAM fp16) -> [128k, 48, nb]
            xt_t = xt_pool.tile([P, KT_ALL, NB], f16, name="xtt")
            xt = xt_t[:, :, :nb]
            nc.sync.dma_start(out=xt[:], in_=x_ext[b0:b0 + nb, :],
                              transpose=True)

        # ---- layer 1: banded matmuls per window pair ----
        hps = {}
        for m in range(N_PAIR):
            ps_t = ps1_pool.tile([P, NB], f32, name="ps1t")
            ps = ps_t[:, :nb]
            mm_list = [(c, t) for c in range(N_CH) for t in _pair_tiles(m)]
            for i, (c, t) in enumerate(mm_list):
                nc.tensor.matmul(
                    ps[:],
                    w1sb[:, BLK_IDX[(m, c, t)], :],
                    xt[:, c * KT_CH + t, :],
                    start=(i == 0),
                    stop=(i == len(mm_list) - 1),
                )
            hp_t = hp_pool.tile([P, NB], f16, name="hpt")
            hp = hp_t[:, :nb]
            if has_bias:
                nc.vector.tensor_scalar_add(hp[:], ps[:], b1sb[:, m:m + 1])
            else:
                nc.vector.tensor_copy(out=hp[:], in_=ps[:])
            hps[m] = hp

        # ---- layer 2: 4 groups of 4 windows ----
        gt_t = gt_pool.tile([P, 2, NB], f16, name="gtt")
        gt = gt_t[:, :, :nb]
        for g in range(4):
            ps2_t = ps2_pool.tile([64, NB], f32, name="ps2t")
            ps2 = ps2_t[:, :nb]
            for piece in range(2):
                nc.tensor.matmul(
                    ps2[:],
                    w2sb[:, 2 * g + piece, :],
                    hps[2 * g + piece][:],
                    start=(piece == 0),
                    stop=(piece == 1),
                )
            lo = 64 * (g % 2)
            if has_bias:
                nc.vector.tensor_scalar_add(
                    gt[lo:lo + 64, g // 2], ps2[:], b2sb[:, g:g + 1],
                )
            else:
                nc.vector.tensor_copy(out=gt[lo:lo + 64, g // 2], in_=ps2[:])

        # ---- layer 3: back to batch-major ----
        osb_t = out_pool.tile([P, NB // P, N_OUT], f32, name="osbt")
        osb = osb_t[:, :nb // P]
        for js in range(nb // P):
            ps3 = ps3_pool.tile([P, N_OUT], f32)
            for piece in range(2):
                nc.tensor.matmul(
                    ps3[:],
                    gt[:, piece, js * P:(js + 1) * P],
                    w3sb[:, piece, :],
                    start=(piece == 0),
                    stop=(piece == 1),
                )
            if has_bias:
                nc.vector.tensor_tensor(
                    osb[:, js], ps3[:], b3sb[:], mybir.AluOpType.add,
                )
            else:
                nc.vector.tensor_copy(out=osb[:, js], in_=ps3[:])
        nc.scalar.dma_start(
            out=out_ext[b0:b0 + nb, :].rearrange("(j p) n -> p j n", p=P),
            in_=osb[:],
        )
        b0 += nb
    if prefetch:
        # prefetch chunks 0/1 for the next iteration (WAR on this iter's reads)
        nc.sync.dma_start(out=xt0[:], in_=x_ext[0:CHUNKS[0], :], transpose=True)
        nc.sync.dma_start(
            out=xt1[:], in_=x_ext[CHUNKS[0]:CHUNKS[0] + CHUNKS[1], :],
            transpose=True)


_CACHED_NC = None


def _prep_in_maps(x, W1, b1, W2, b2, W3, b3):
    x16 = np.asarray(x, dtype=np.float16)
    w1sb, w2sb, w3sb, b1t, b2t, b3t = _pack_weights(W1, b1, W2, b2, W3, b3)
    in_maps = []
    for i in range(N_CORES):
        in_maps.append({
            "x": x16[i * B_SHARD:(i + 1) * B_SHARD],
            "w1": w1sb,
            "w2": w2sb,
            "w3": w3sb,
            "b1t": b1t,
            "b2t": b2t,
            "b3t": b3t,
        })
    return in_maps


_CACHED_BIAS_NC = None


def kernel(x, W1, b1, W2, b2, W3, b3):
    global _CACHED_NC, _CACHED_BIAS_NC
    has_bias = bool(
        np.any(np.asarray(b1)) or np.any(np.asarray(b2)) or np.any(np.asarray(b3))
    )
    if has_bias:
        if _CACHED_BIAS_NC is None:
            _CACHED_BIAS_NC = build_kernel(has_bias=True)
        nc = _CACHED_BIAS_NC
    else:
        if _CACHED_NC is None:
            _CACHED_NC = build_kernel()
        nc = _CACHED_NC
    in_maps = _prep_in_maps(x, W1, b1, W2, b2, W3, b3)
    last_err = None
    for attempt in range(3):
        try:
            res = run_bass_kernel_spmd(nc, in_maps, core_ids=list(range(N_CORES)))
            break
        except Exception as e:  # transient device/axon failures
            last_err = e
            if attempt == 2:
                raise
            import time as _time
            _time.sleep(20.0)
    return np.concatenate([res.results[i]["out"] for i in range(N_CORES)], axis=0)

